# revision 1
# baseline (speedup 1.0000x reference)
"""GNN aggregator (NGCF-style) Trainium2 kernel.

y = LeakyReLU((ego + A@ego) @ W1 + b1) + LeakyReLU((ego * (A@ego)) @ W2 + b2)

where A@ego is an edge-list SpMM: side[dst] += w_e * ego[src_e].

Strategy (8 NeuronCores, SPMD single NEFF, no collectives):
  - Destination nodes are partitioned across the 8 cores (12500 each); each
    core computes its output rows independently from a full copy of the
    (bf16) embedding table in its HBM.
  - Host packs each core's nodes into 128-node "blocks". Per block, its
    edges are grouped into 4 source-range buckets (dma_gather indices are
    int16, so a gather window is <= 32767 rows; buckets are 25000 rows).
    Each (block, bucket) run is padded to a whole number of 128-edge tiles
    with a compile-time fixed tile capacity (template rotated per block so
    all cores share one instruction stream).
  - Device: for each group of 8 blocks, 4 dma_gather instructions (one per
    bucket, spanning the group) pull source rows into SBUF as [e, f] tiles,
    rotated across the 4 SWDGE queues (4 Q7 core pairs generate and drain
    descriptors concurrently: measured ~2.2 ns/descriptor vs ~28 ns on a
    single queue).
    For each 128-edge tile, one DVE tensor_scalar builds
    S[e, d] = w_e * (d == dst_local_e), and one PE matmul accumulates
    side_T[f, d] += G[e, f].T @ S[e, d] into a PSUM accumulator.
  - PSUM accumulation groups have 2KB zero-region (bank) granularity, so 4
    block accumulators share one bank and the bank is opened by a single
    full-width zeroing matmul (start=True); all real matmuls accumulate
    (start=False) and are ordered after it via the WAW dep on the tile.
  - Finals per block (fp32): sumT = egoT + side_T ; biT = egoT * side_T ;
    out1_T = W1.T @ sumT (one matmul per private PSUM bank); leaky (+bias)
    on DVE; y_T = leaky1 + leaky2 -> DMA out transposed; host unpermutes.
"""

import math
from dataclasses import dataclass, field

import ml_dtypes
import numpy as np

# ----------------------------------------------------------------------------
# problem constants (hardcoded; kernel.py must be self-contained)
# ----------------------------------------------------------------------------
N = 100000
E = 1600000
D = 128
NCORES = 8
NEG_SLOPE = 0.01
P = 128

BF16 = ml_dtypes.bfloat16


# ----------------------------------------------------------------------------
# compile-time template
# ----------------------------------------------------------------------------
@dataclass(frozen=True)
class Cfg:
    n_nodes: int = N
    n_cores: int = NCORES
    bucket: int = 25000            # gather window rows (< 32768)
    caps: tuple = (5, 5, 4, 4)     # tiles per (block, bucket), rotated by block
    group: int = 8                 # blocks per group
    n_blocks: int = 98             # blocks per core
    with_bias: bool = False        # emit bias-add ops (b1/b2 nonzero)
    rounds: int = 1                # repeat whole pipeline (benchmarking only)
    chunk_tiles: int = 0           # gather chunk size (tiles); 0 = unchunked
    deep_bufs: bool = True         # deeper double-buffering for DMA/compute overlap

    @property
    def nodes_per_core(self):
        assert self.n_nodes % self.n_cores == 0
        return self.n_nodes // self.n_cores

    @property
    def n_buckets(self):
        return math.ceil(self.n_nodes / self.bucket)

    def cap(self, j, k):
        return self.caps[(k + j) % len(self.caps)]


@dataclass
class Template:
    """Compile-time structure shared by host prep and device build."""
    cfg: Cfg
    groups: list = field(default_factory=list)        # list[list[block]]
    tile_block: list = field(default_factory=list)    # tile -> block j
    tile_last: list = field(default_factory=list)     # tile is block's last
    gathers: list = field(default_factory=list)       # (g, k, tile_start, n_tiles)
    run_start: dict = field(default_factory=dict)     # (j, k) -> start tile
    group_tile_off: list = field(default_factory=list)  # group -> first tile
    total_tiles: int = 0


def build_template(cfg: Cfg) -> Template:
    t = Template(cfg)
    nb = cfg.n_buckets
    assert len(cfg.caps) == nb
    assert cfg.bucket <= 32767
    blocks = list(range(cfg.n_blocks))
    t.groups = [blocks[i:i + cfg.group] for i in range(0, cfg.n_blocks, cfg.group)]
    last_tile = {}
    for g, bl in enumerate(t.groups):
        t.group_tile_off.append(len(t.tile_block))
        for k in range(nb):
            start = len(t.tile_block)
            for j in bl:
                t.run_start[(j, k)] = len(t.tile_block)
                for _ in range(cfg.cap(j, k)):
                    ti = len(t.tile_block)
                    t.tile_block.append(j)
                    last_tile[j] = ti
            t.gathers.append((g, k, start, len(t.tile_block) - start))
    t.total_tiles = len(t.tile_block)
    t.tile_last = [last_tile[j] == i for i, j in enumerate(t.tile_block)]
    return t


# ----------------------------------------------------------------------------
# host-side packing and data prep
# ----------------------------------------------------------------------------
def _greedy_pack(deg, cfg: Cfg):
    """Assign nodes (rows of deg [n, n_buckets]) to blocks subject to
    per-(block,bucket) edge capacity and 128 slots per block.
    Returns block_of [n] or None on failure."""
    n = deg.shape[0]
    nb = cfg.n_buckets
    B = cfg.n_blocks
    caps = np.array([[cfg.cap(j, k) * P for k in range(nb)] for j in range(B)],
                    dtype=np.int64)
    rem = caps.copy()
    rem_slots = np.full(B, P, dtype=np.int64)
    order = np.argsort(-deg.sum(1), kind="stable")
    block_of = np.full(n, -1, dtype=np.int64)
    for idx in order:
        d = deg[idx]
        feas = (rem_slots > 0) & np.all(rem >= d, axis=1)
        if not feas.any():
            return None
        # balance: maximize the minimum relative slack after placement
        slack = (rem - d) / caps
        score = slack.min(axis=1) + 0.001 * rem_slots
        score[~feas] = -np.inf
        j = int(np.argmax(score))
        block_of[idx] = j
        rem[j] -= d
        rem_slots[j] -= 1
    return block_of


def host_prep(inputs, cfg: Cfg, tmpl: Template):
    """Build per-core input dicts + metadata for output assembly."""
    ego = np.ascontiguousarray(inputs["ego_embeddings"], dtype=np.float32)
    ew = np.asarray(inputs["edge_weight"], dtype=np.float32)
    W1 = np.ascontiguousarray(inputs["W1"], dtype=np.float32)
    b1 = np.asarray(inputs["b1"], dtype=np.float32)
    W2 = np.ascontiguousarray(inputs["W2"], dtype=np.float32)
    b2 = np.asarray(inputs["b2"], dtype=np.float32)
    es = np.asarray(inputs["edge_src"]).astype(np.int64)
    ed = np.asarray(inputs["edge_dst"]).astype(np.int64)

    npc = cfg.nodes_per_core
    nb = cfg.n_buckets
    T = tmpl.total_tiles

    table_bf16 = ego.astype(BF16)
    import os as _os
    if _os.environ.get("STRIDE_IOTA", "0") == "1":
        iota = np.zeros((P, 2 * P), dtype=BF16)
        iota[:, ::2] = np.arange(P, dtype=np.float32).astype(BF16)[None, :]
    else:
        iota = np.broadcast_to(np.arange(P, dtype=np.float32), (P, P)).astype(BF16)
        iota = np.ascontiguousarray(iota)
    b1col = np.ascontiguousarray(b1[:, None])
    b2col = np.ascontiguousarray(b2[:, None])

    core_of_edge = ed // npc
    bucket_of_edge = es // cfg.bucket

    in_maps = []
    node_maps = []
    for c in range(cfg.n_cores):
        m = core_of_edge == c
        src_c = es[m]
        dstl_c = ed[m] - c * npc
        w_c = ew[m]
        bkt_c = bucket_of_edge[m]

        deg = np.bincount(dstl_c * nb + bkt_c, minlength=npc * nb).reshape(npc, nb)
        block_of = _greedy_pack(deg, cfg)
        if block_of is None:
            raise RuntimeError(f"packing failed for core {c} with caps {cfg.caps}")
        # slots within block: stable order by block
        order_nodes = np.argsort(block_of, kind="stable")
        slot_of = np.empty(npc, dtype=np.int64)
        blk_sorted = block_of[order_nodes]
        first_idx = np.searchsorted(blk_sorted, np.arange(cfg.n_blocks))
        slot_of[order_nodes] = np.arange(npc) - first_idx[blk_sorted]
        assert slot_of.max() < P

        node_map = np.full(cfg.n_blocks * P, -1, dtype=np.int64)
        node_map[block_of * P + slot_of] = np.arange(npc) + c * npc
        node_maps.append(node_map)

        # per-edge placement into the padded tile stream
        eblk = block_of[dstl_c]
        key = eblk * nb + bkt_c
        order_e = np.argsort(key, kind="stable")
        key_s = key[order_e]
        run_ids, run_firsts = np.unique(key_s, return_index=True)
        rank = np.arange(len(key_s))
        rank = rank - run_firsts[np.searchsorted(run_ids, key_s)]
        run_start_flat = np.full(cfg.n_blocks * nb, -1, dtype=np.int64)
        for (j, k), st in tmpl.run_start.items():
            run_start_flat[j * nb + k] = st
        counts = np.bincount(key_s, minlength=cfg.n_blocks * nb)
        caps_flat = np.array([cfg.cap(j, k) * P for j in range(cfg.n_blocks)
                              for k in range(nb)], dtype=np.int64)
        if (counts > caps_flat).any():
            bad = int(np.argmax(counts - caps_flat))
            raise RuntimeError(
                f"core {c}: run {bad // nb},{bad % nb} overflow "
                f"{counts[bad]} > {caps_flat[bad]}")
        pos = run_start_flat[key_s] * P + rank
        assert pos.max() < T * P

        idx_flat = np.zeros(T * P, dtype=np.int16)
        dst_flat = np.zeros(T * P, dtype=np.float32)
        w_flat = np.zeros(T * P, dtype=np.float32)
        idx_flat[pos] = (src_c[order_e] - bkt_c[order_e] * cfg.bucket).astype(np.int16)
        dst_flat[pos] = slot_of[dstl_c[order_e]].astype(np.float32)
        w_flat[pos] = w_c[order_e]

        idx_cols = idx_flat.reshape(-1, 16).T  # [16, T*8]
        idx_h = np.ascontiguousarray(np.tile(idx_cols, (8, 1)))  # [128, T*8]
        stream_h = np.ascontiguousarray(
            np.stack([dst_flat.reshape(T, P).T, w_flat.reshape(T, P).T],
                     axis=-1))  # [128, T, 2] f32
        if _os.environ.get("S2OP", "0") == "1":
            stream_h = stream_h.astype(BF16)

        egoT = np.zeros((P, cfg.n_blocks * P), dtype=np.float32)
        valid = node_map >= 0
        egoT[:, valid] = ego[node_map[valid]].T

        im = {
            "table": table_bf16,
            "idxs": idx_h,
            "stream": stream_h,
            "egoT": egoT,
            "w1": W1,   # [in=f, out] == lhsT [K=f, M=dout]
            "w2": W2,
            "iota": iota,
        }
        if cfg.with_bias:
            im["b1col"] = b1col
            im["b2col"] = b2col
        in_maps.append(im)
    return in_maps, node_maps


def assemble_output(results, node_maps, cfg: Cfg):
    y = np.zeros((cfg.n_nodes, D), dtype=np.float32)
    for c in range(cfg.n_cores):
        yT = results[c]["yT"]  # [128, n_blocks*P]
        nm = node_maps[c]
        valid = nm >= 0
        y[nm[valid]] = yT[:, valid].T
    return y


# ----------------------------------------------------------------------------
# device kernel
# ----------------------------------------------------------------------------
def _patch_sem_cleanup():
    """The walrus build in this container rejects the
    EVENT_SEMAPHORE_RANGE_CLEAR InstISA ("ISA wrong length") that
    TileContext emits on exit via Bass.clear_and_free_semaphores. The
    cleanup only matters for multi-iteration NEFFs (NRT re-initializes
    semaphores per execution), so skip the instruction emission and keep
    the allocator bookkeeping."""
    import concourse.bass as bass

    if getattr(bass.Bass, "_sem_cleanup_patched", False):
        return

    def patched(self, sems):
        if not sems:
            return
        sem_nums = [s.num if hasattr(s, "num") else s for s in sems]
        self._state.prepend_free_semaphores(sem_nums)
        for poison_set in self._tile_sem_poison_stack:
            poison_set.update(sem_nums)

    bass.Bass.clear_and_free_semaphores = patched
    bass.Bass._sem_cleanup_patched = True


_MANY_WAITS_OK = {
    "InstEventSemaphore",
}


def _split_excess_waits(nc, mybir, max_waits=1):
    """This container's walrus encodes at most `max_waits` sync-wait commands
    on TPB compute instructions ("Too many sync wait commands"). Hoist the
    excess onto EventSemaphore instructions inserted immediately before on
    the same engine — semantically identical (in-order sequencer stall)."""
    nid = 0
    for blk in nc.m.functions[0].blocks:
        il = blk.instructions
        i = 0
        while i < len(il):
            ins = il[i]
            si = ins.sync_info
            if (type(ins).__name__ not in _MANY_WAITS_OK and si is not None
                    and si.on_wait and len(si.on_wait) > max_waits):
                waits = list(si.on_wait)
                excess, keep = waits[:-max_waits], waits[-max_waits:]
                ins.sync_info = mybir.SyncInfo(
                    on_wait=keep, on_update=list(si.on_update or []))
                for w in excess:
                    es = mybir.InstEventSemaphore(
                        name=f"I-waitsplit-{nid}", engine=ins.engine,
                        ins=[], outs=[],
                        sync_info=mybir.SyncInfo(on_wait=[w], on_update=[]))
                    nid += 1
                    il.insert(i, es)
                    i += 1
            i += 1


def build_nc(cfg: Cfg, tmpl: Template):
    import concourse.bass as bass
    import concourse.mybir as mybir
    from concourse import library_config
    from concourse.tile import TileContext

    _patch_sem_cleanup()

    dt = mybir.dt
    T = tmpl.total_tiles
    NBLK = cfg.n_blocks

    nc = bass.Bass(num_swdge_queues=4)
    table = nc.dram_tensor("table", [cfg.n_nodes, D], dt.bfloat16, kind="ExternalInput")
    idxs = nc.dram_tensor("idxs", [P, T * 8], dt.int16, kind="ExternalInput")
    import os as _os
    _s2op = _os.environ.get("S2OP", "0") == "1"
    _s2w = _os.environ.get("S2W", "0") == "1"
    stream = nc.dram_tensor("stream", [P, T, 2],
                            dt.bfloat16 if _s2op else dt.float32,
                            kind="ExternalInput")
    egoT = nc.dram_tensor("egoT", [P, NBLK * P], dt.float32, kind="ExternalInput")
    w1 = nc.dram_tensor("w1", [D, D], dt.float32, kind="ExternalInput")
    w2 = nc.dram_tensor("w2", [D, D], dt.float32, kind="ExternalInput")
    import os as _os
    _stride_iota = _os.environ.get("STRIDE_IOTA", "0") == "1"
    iota = nc.dram_tensor("iota", [P, 2 * P] if _stride_iota else [P, P],
                          dt.bfloat16, kind="ExternalInput")
    if cfg.with_bias:
        b1col = nc.dram_tensor("b1col", [D, 1], dt.float32, kind="ExternalInput")
        b2col = nc.dram_tensor("b2col", [D, 1], dt.float32, kind="ExternalInput")
    yT = nc.dram_tensor("yT", [P, NBLK * P], dt.float32, kind="ExternalOutput")

    nc.gpsimd.load_library(library_config.mlp)

    with TileContext(nc) as tc:
        with (
            tc.tile_pool(name="const", bufs=1) as constp,
            tc.tile_pool(name="stage", bufs=3 if cfg.deep_bufs else 2) as stagep,
            tc.tile_pool(name="idxp", bufs=3 if cfg.deep_bufs else 2) as idxp,
            tc.tile_pool(name="streamp", bufs=3 if cfg.deep_bufs else 2) as streamp,
            tc.tile_pool(name="egop", bufs=3 if cfg.deep_bufs else 2) as egop,
            tc.tile_pool(name="sp", bufs=16 if cfg.deep_bufs else 8) as sp,
            tc.tile_pool(name="finp", bufs=16 if cfg.deep_bufs else 12) as finp,
            tc.tile_pool(name="outp", bufs=2) as outp,
            tc.tile_pool(name="accp", bufs=4, space="PSUM") as accp,
            tc.tile_pool(name="fpsum", bufs=2, space="PSUM") as fpsump,
        ):
            w1t = constp.tile([D, D], dt.float32)
            nc.sync.dma_start(out=w1t[:], in_=w1[:, :])
            w2t = constp.tile([D, D], dt.float32)
            nc.sync.dma_start(out=w2t[:], in_=w2[:, :])
            iotat_full = constp.tile([P, 2 * P] if _stride_iota else [P, P],
                                     dt.bfloat16, name="iotat_full")
            nc.sync.dma_start(out=iotat_full[:], in_=iota[:, :])
            if _stride_iota:
                iotat = iotat_full.rearrange("p (d two) -> p d two", two=2)[:, :, 0]
                iota_sq = iotat_full[:, 0:P]  # contiguous view for zero-matmul lhsT
            else:
                iotat = iotat_full
                iota_sq = iotat_full
            zerot = constp.tile([P, 4 * P], dt.bfloat16)
            nc.vector.memset(zerot[:], 0.0)
            if cfg.with_bias:
                b1t = constp.tile([D, 1], dt.float32)
                nc.sync.dma_start(out=b1t[:], in_=b1col[:, :])
                b2t = constp.tile([D, 1], dt.float32)
                nc.sync.dma_start(out=b2t[:], in_=b2col[:, :])

            gather_by_group = {}
            for (g, k, ts0, ntk) in tmpl.gathers:
                gather_by_group.setdefault(g, []).append((k, ts0, ntk))

            rounds = getattr(cfg, "rounds", 1)
            _gq = [0]  # global gather chunk counter for queue rotation
            # one register per distinct gather size (to_reg per call would
            # exhaust the Pool engine's register file)
            nidx_regs = {}
            for (_, _, _, ntk) in tmpl.gathers:
                n_idx = ntk * P
                if n_idx not in nidx_regs:
                    nidx_regs[n_idx] = nc.gpsimd.to_reg(n_idx)

            group_sched = [(r, g, bl) for r in range(rounds)
                           for g, bl in enumerate(tmpl.groups)]
            for _round, g, bl in group_sched:
                goff = tmpl.group_tile_off[g]
                gend = (tmpl.group_tile_off[g + 1]
                        if g + 1 < len(tmpl.groups) else T)
                ntg = gend - goff
                nblg = len(bl)

                idx_t = idxp.tile([P, ntg * 8], dt.int16, tag="idx")
                nc.sync.dma_start(out=idx_t[:], in_=idxs[:, goff * 8:gend * 8])
                str_t = streamp.tile([P, ntg, 2],
                                     dt.bfloat16 if _s2op else dt.float32,
                                     tag="stream")
                nc.sync.dma_start(out=str_t[:], in_=stream[:, goff:gend, :])
                ego_t = egop.tile([P, nblg * P], dt.float32, tag="ego")
                nc.sync.dma_start(
                    out=ego_t[:], in_=egoT[:, bl[0] * P:(bl[-1] + 1) * P])

                stage_t = stagep.tile([P, ntg, D], dt.bfloat16, tag="stage")
                # Chunk each (group,bucket) gather to <=1024 idxs with
                # single_packet=True and rotate the 4 SWDGE queues: measured
                # ~2.2 ns/descriptor vs ~28 ns single-queue/packetized.
                glist = gather_by_group[g]
                # rotate issue order per group so desc-gen latency doesn't
                # always delay the same queue's drain
                glist = glist[g % len(glist):] + glist[:g % len(glist)]
                for (k, ts0, ntk) in glist:
                    blo = k * cfg.bucket
                    bhi = min((k + 1) * cfg.bucket, cfg.n_nodes)
                    done = 0
                    chunk = cfg.chunk_tiles if cfg.chunk_tiles > 0 else ntk
                    while done < ntk:
                        cn = min(chunk, ntk - done)
                        loc = ts0 - goff + done
                        n_idx = cn * P
                        if n_idx not in nidx_regs:
                            nidx_regs[n_idx] = nc.gpsimd.to_reg(n_idx)
                        nc.gpsimd.dma_gather(
                            stage_t[:, loc:loc + cn, :],
                            table[blo:bhi, :],
                            idx_t[:, loc * 8:(loc + cn) * 8],
                            n_idx,
                            nidx_regs[n_idx],
                            D,
                            single_packet=n_idx <= 1024,
                            queue_num=_gq[0] % 4,
                        )
                        _gq[0] += 1
                        done += cn

                # aggregation matmuls (stream order)
                n_quads = (nblg + 3) // 4
                quads = [accp.tile([P, 4, P], dt.float32, tag="acc",
                                   name=f"acc_g{g}_q{q}")
                         for q in range(n_quads)]
                # open each quad bank: one zeroing matmul covering all 4
                # block regions, so every later matmul is a plain accumulate
                # ordered after it (WAW on the full tile).
                for q in range(n_quads):
                    nc.tensor.matmul(
                        out=quads[q][:, :, :],
                        lhsT=iota_sq[:],
                        rhs=zerot[:],
                        start=True,
                        stop=False,
                        skip_group_check=True,
                    )

                def acc_ap(jj):
                    return quads[jj // 4][:, jj % 4, :]

                if _s2w:
                    # build S for the whole group in 2 wide DVE ops per half
                    # (instruction-count-bound: ~200ns/op overhead dominates)
                    swide = sp.tile([P, ntg, P], dt.bfloat16, tag="Swide",
                                    name=f"swide_{_round}_{g}", bufs=2)
                    half = (ntg + 1) // 2
                    for h0 in range(0, ntg, half):
                        h1 = min(h0 + half, ntg)
                        iota3 = iotat.rearrange("p (one d) -> p one d", one=1).to_broadcast(
                            [P, h1 - h0, P])
                        nc.vector.tensor_tensor(
                            out=swide[:, h0:h1, :], in0=iota3,
                            in1=str_t[:, h0:h1, 0:1].to_broadcast(
                                [P, h1 - h0, P]),
                            op=mybir.AluOpType.is_equal)
                        nc.vector.tensor_tensor(
                            out=swide[:, h0:h1, :], in0=swide[:, h0:h1, :],
                            in1=str_t[:, h0:h1, 1:2].to_broadcast(
                                [P, h1 - h0, P]),
                            op=mybir.AluOpType.mult)

                import os as _os
                _ablate_s = _os.environ.get("ABLATE_S", "0") == "1"
                for ti in range(goff, gend):
                    j = tmpl.tile_block[ti]
                    jj = j - bl[0]
                    loc_t = ti - goff
                    if _s2w:
                        S = swide[:, loc_t, :]
                        pass
                    elif False:
                        pass
                    else:
                        S = sp.tile([P, P], dt.bfloat16, tag="S")
                    if (not _s2w) and _s2op:
                        eq = sp.tile([P, P], dt.bfloat16, tag="Seq")
                        nc.vector.tensor_tensor(
                            out=eq[:], in0=iotat[:],
                            in1=str_t[:, loc_t, 0:1].to_broadcast([P, P]),
                            op=mybir.AluOpType.is_equal)
                        nc.vector.tensor_tensor(
                            out=S[:], in0=eq[:],
                            in1=str_t[:, loc_t, 1:2].to_broadcast([P, P]),
                            op=mybir.AluOpType.mult)
                    elif not _s2w:
                        nc.vector.tensor_scalar(
                            out=S[:, 0:1] if _ablate_s else S[:],
                            in0=iotat[:, 0:1] if _ablate_s else iotat[:],
                            scalar1=str_t[:, loc_t, 0:1],
                            scalar2=str_t[:, loc_t, 1:2],
                            op0=mybir.AluOpType.is_equal,
                            op1=mybir.AluOpType.mult,
                        )
                    nc.tensor.matmul(
                        out=acc_ap(jj),
                        lhsT=stage_t[:, loc_t, :],
                        rhs=S[:],
                        start=False,
                        stop=tmpl.tile_last[ti],
                        skip_group_check=True,
                    )

                # finals
                out_t = outp.tile([P, nblg * P], dt.float32, tag="out")
                for jj, j in enumerate(bl):
                    a = acc_ap(jj)
                    ego_j = ego_t[:, jj * P:(jj + 1) * P]
                    sumT = finp.tile([P, P], dt.float32, tag="sumT")
                    nc.vector.tensor_add(out=sumT[:], in0=ego_j, in1=a)
                    biT = finp.tile([P, P], dt.float32, tag="biT")
                    nc.vector.tensor_tensor(
                        out=biT[:], in0=ego_j, in1=a, op=mybir.AluOpType.mult)
                    pp1 = fpsump.tile([P, P], dt.float32, tag="pp1")
                    nc.tensor.matmul(out=pp1[:], lhsT=w1t[:], rhs=sumT[:],
                                     start=True, stop=True,
                                     skip_group_check=True)
                    pp2 = fpsump.tile([P, P], dt.float32, tag="pp2")
                    nc.tensor.matmul(out=pp2[:], lhsT=w2t[:], rhs=biT[:],
                                     start=True, stop=True,
                                     skip_group_check=True)
                    if cfg.with_bias:
                        z1 = finp.tile([P, P], dt.float32, tag="z1")
                        nc.vector.tensor_scalar(
                            out=z1[:], in0=pp1[:], scalar1=b1t[:, 0:1],
                            scalar2=None, op0=mybir.AluOpType.add)
                        z2 = finp.tile([P, P], dt.float32, tag="z2")
                        nc.vector.tensor_scalar(
                            out=z2[:], in0=pp2[:], scalar1=b2t[:, 0:1],
                            scalar2=None, op0=mybir.AluOpType.add)
                    else:
                        z1, z2 = pp1, pp2
                    t1 = finp.tile([P, P], dt.float32, tag="t1")
                    nc.vector.tensor_scalar(
                        out=t1[:], in0=z1[:], scalar1=NEG_SLOPE,
                        scalar2=None, op0=mybir.AluOpType.mult)
                    m1 = finp.tile([P, P], dt.float32, tag="m1")
                    nc.vector.tensor_tensor(
                        out=m1[:], in0=z1[:], in1=t1[:],
                        op=mybir.AluOpType.max)
                    t2 = finp.tile([P, P], dt.float32, tag="t2")
                    nc.vector.tensor_scalar(
                        out=t2[:], in0=z2[:], scalar1=NEG_SLOPE,
                        scalar2=None, op0=mybir.AluOpType.mult)
                    m2 = finp.tile([P, P], dt.float32, tag="m2")
                    nc.vector.tensor_tensor(
                        out=m2[:], in0=z2[:], in1=t2[:],
                        op=mybir.AluOpType.max)
                    nc.vector.tensor_add(
                        out=out_t[:, jj * P:(jj + 1) * P], in0=m1[:], in1=m2[:])

                nc.sync.dma_start(
                    out=yT[:, bl[0] * P:(bl[-1] + 1) * P], in_=out_t[:])

    return nc


def finalize_for_hw(nc):
    """Walrus-compat passes applied only on the compile path (CoreSim's
    race detector rejects the post-hoc EventSemaphore instructions)."""
    import concourse.mybir as mybir
    if getattr(nc, "_finalized_for_hw", False):
        return nc
    # raw-Bass doesn't run Bacc's codegen pass that fills in .instr bytes for
    # extended-ISA instructions (load_library pseudo); without it walrus
    # fails with "ISA wrong length".
    mybir.codegen_inst_isa_subclasses(nc)
    _split_excess_waits(nc, mybir)
    nc._finalized_for_hw = True
    return nc


# ----------------------------------------------------------------------------
# entry point
# ----------------------------------------------------------------------------
_CACHE = {}
LAST_EXEC_NS = None
TRACE = False

CAPS_LADDER = [(5, 4, 4, 4), (5, 5, 4, 4), (5, 5, 5, 4), (5, 5, 5, 5), (6, 5, 5, 5),
               (6, 6, 5, 5), (6, 6, 6, 6), (8, 8, 8, 8), (12, 12, 12, 12)]


def _get_compiled(cfg: Cfg):
    if cfg not in _CACHE:
        tmpl = build_template(cfg)
        nc = build_nc(cfg, tmpl)
        _CACHE[cfg] = (tmpl, nc)
    return _CACHE[cfg]


def kernel(**inputs) -> np.ndarray:
    global LAST_EXEC_NS
    with_bias = (np.any(np.asarray(inputs["b1"]) != 0)
                 or np.any(np.asarray(inputs["b2"]) != 0))
    chosen = None
    in_maps = node_maps = None
    for caps in CAPS_LADDER:
        trial = Cfg(caps=caps, with_bias=bool(with_bias))
        tmpl_try = build_template(trial)
        try:
            in_maps, node_maps = host_prep(inputs, trial, tmpl_try)
        except RuntimeError:
            continue
        chosen = trial
        break
    if chosen is None:
        raise RuntimeError("no feasible caps template")

    tmpl, nc = _get_compiled(chosen)
    finalize_for_hw(nc)

    from concourse.bass_utils import run_bass_kernel_spmd
    res = run_bass_kernel_spmd(
        nc, in_maps, core_ids=list(range(chosen.n_cores)), trace=TRACE)
    LAST_EXEC_NS = res.exec_time_ns
    return assemble_output(res.results, node_maps, chosen)



# revision 13
# speedup vs baseline: 34.1586x; 34.1586x over previous
"""GNN aggregator (NGCF-style) Trainium2 kernel.

y = LeakyReLU((ego + A@ego) @ W1 + b1) + LeakyReLU((ego * (A@ego)) @ W2 + b2)

where A@ego is an edge-list SpMM: side[dst] += w_e * ego[src_e].

Strategy (8 NeuronCores, SPMD single NEFF, no collectives):
  - Destination nodes are partitioned across the 8 cores (12500 each); each
    core computes its output rows independently from a full copy of the
    (bf16) embedding table in its HBM.
  - Host packs each core's nodes into 128-node "blocks". Per block, its
    edges are grouped into 4 source-range buckets (dma_gather indices are
    int16, so a gather window is <= 32767 rows; buckets are 25000 rows).
    Each (block, bucket) run is padded to a whole number of 128-edge tiles
    with a compile-time fixed tile capacity (template rotated per block so
    all cores share one instruction stream).
  - Device: for each group of 8 blocks, 4 dma_gather instructions (one per
    bucket, spanning the group) pull source rows into SBUF as [e, f] tiles,
    rotated across the 4 SWDGE queues.
  - S-build (s2w): for each group, wide DVE tensor_tensor ops build the
    whole selection-matrix stack S[e, t, d] = w_(e,t) * (d == dst_(e,t))
    in a few chunks (vs one small op per tile, which serializes PE on
    per-tile semaphore waits and pays 45 ns DVE seq overhead per tile).
    With the (dst, w) stream in bf16 (s2op) the DVE runs in its 2x 16-bit
    mode.
  - Per 128-edge tile, one PE matmul accumulates
    side_T[f, d] += G[e, f].T @ S[e, d] into a PSUM accumulator; 4 block
    accumulators share one PSUM bank (opened by a single full-width zeroing
    matmul with start=True; all real matmuls accumulate with start=False,
    ordered after it by the WAW dep on the tile).
  - Finals quad-wide (quad_fin): for each 4-block quad, the PSUM bank is
    one [P, 512] accumulator view: sumT = egoT + side_T and
    biT = egoT * side_T as two wide DVE ops; one f32r matmul per branch
    (f32r runs 1 cycle/row at free dim >= 256, 4x faster than plain fp32)
    into its own PSUM bank; LeakyReLU (+bias) on the Activation engine
    straight out of PSUM (act_lrelu); final add on DVE -> DMA out
    transposed; host unpermutes.
"""

import math
from dataclasses import dataclass, field

import ml_dtypes
import numpy as np

# ----------------------------------------------------------------------------
# problem constants (hardcoded; kernel.py must be self-contained)
# ----------------------------------------------------------------------------
N = 100000
E = 1600000
D = 128
NCORES = 8
NEG_SLOPE = 0.01
P = 128

BF16 = ml_dtypes.bfloat16


# ----------------------------------------------------------------------------
# compile-time template
# ----------------------------------------------------------------------------
@dataclass(frozen=True)
class Cfg:
    n_nodes: int = N
    n_cores: int = NCORES
    bucket: int = 25000            # gather window rows (< 32768)
    caps: tuple = (5, 5, 4, 4)     # tiles per (block, bucket), rotated by block
    group: int = 8                 # blocks per group
    n_blocks: int = 98             # blocks per core
    with_bias: bool = False        # emit bias-add ops (b1/b2 nonzero)
    rounds: int = 1                # repeat whole pipeline (benchmarking only)
    chunk_tiles: int = 0           # gather chunk size (tiles); 0 = unchunked
    deep_bufs: bool = True         # deeper double-buffering for DMA/compute overlap
    s2w: bool = True               # wide S-build per group (vs per-tile ops)
    s2op: bool = True              # (dst, w) stream in bf16 (vs f32)
    s_chunks: int = 2              # wide S-build DVE chunks per group
    quad_fin: bool = True          # finals quad-wide [P, 512] (vs per-block)
    f32r: bool = True              # finals matmuls in f32r (1 cyc/row at >=256)
    act_lrelu: bool = True         # LeakyReLU on Activation engine from PSUM
    ablate_gather: bool = False    # benchmarking only: skip dma_gather
    ablate_mm: bool = False        # benchmarking only: skip aggregation matmuls
    ablate_s: bool = False         # benchmarking only: skip S-build (rhs=iota)

    @property
    def nodes_per_core(self):
        assert self.n_nodes % self.n_cores == 0
        return self.n_nodes // self.n_cores

    @property
    def n_buckets(self):
        return math.ceil(self.n_nodes / self.bucket)

    def cap(self, j, k):
        return self.caps[(k + j) % len(self.caps)]


@dataclass
class Template:
    """Compile-time structure shared by host prep and device build."""
    cfg: Cfg
    groups: list = field(default_factory=list)        # list[list[block]]
    tile_block: list = field(default_factory=list)    # tile -> block j
    tile_last: list = field(default_factory=list)     # tile is block's last
    gathers: list = field(default_factory=list)       # (g, k, tile_start, n_tiles)
    run_start: dict = field(default_factory=dict)     # (j, k) -> start tile
    group_tile_off: list = field(default_factory=list)  # group -> first tile
    total_tiles: int = 0


def build_template(cfg: Cfg) -> Template:
    t = Template(cfg)
    nb = cfg.n_buckets
    assert len(cfg.caps) == nb
    assert cfg.bucket <= 32767
    blocks = list(range(cfg.n_blocks))
    t.groups = [blocks[i:i + cfg.group] for i in range(0, cfg.n_blocks, cfg.group)]
    last_tile = {}
    for g, bl in enumerate(t.groups):
        t.group_tile_off.append(len(t.tile_block))
        for k in range(nb):
            start = len(t.tile_block)
            for j in bl:
                t.run_start[(j, k)] = len(t.tile_block)
                for _ in range(cfg.cap(j, k)):
                    ti = len(t.tile_block)
                    t.tile_block.append(j)
                    last_tile[j] = ti
            t.gathers.append((g, k, start, len(t.tile_block) - start))
    t.total_tiles = len(t.tile_block)
    t.tile_last = [last_tile[j] == i for i, j in enumerate(t.tile_block)]
    return t


# ----------------------------------------------------------------------------
# host-side packing and data prep
# ----------------------------------------------------------------------------
def _greedy_pack(deg, cfg: Cfg):
    """Assign nodes (rows of deg [n, n_buckets]) to blocks subject to
    per-(block,bucket) edge capacity and 128 slots per block.
    Returns block_of [n] or None on failure."""
    n = deg.shape[0]
    nb = cfg.n_buckets
    B = cfg.n_blocks
    caps = np.array([[cfg.cap(j, k) * P for k in range(nb)] for j in range(B)],
                    dtype=np.int64)
    rem = caps.copy()
    rem_slots = np.full(B, P, dtype=np.int64)
    order = np.argsort(-deg.sum(1), kind="stable")
    block_of = np.full(n, -1, dtype=np.int64)
    for idx in order:
        d = deg[idx]
        feas = (rem_slots > 0) & np.all(rem >= d, axis=1)
        if not feas.any():
            return None
        # balance: maximize the minimum relative slack after placement
        slack = (rem - d) / caps
        score = slack.min(axis=1) + 0.001 * rem_slots
        score[~feas] = -np.inf
        j = int(np.argmax(score))
        block_of[idx] = j
        rem[j] -= d
        rem_slots[j] -= 1
    return block_of


def host_prep(inputs, cfg: Cfg, tmpl: Template):
    """Build per-core input dicts + metadata for output assembly."""
    ego = np.ascontiguousarray(inputs["ego_embeddings"], dtype=np.float32)
    ew = np.asarray(inputs["edge_weight"], dtype=np.float32)
    W1 = np.ascontiguousarray(inputs["W1"], dtype=np.float32)
    b1 = np.asarray(inputs["b1"], dtype=np.float32)
    W2 = np.ascontiguousarray(inputs["W2"], dtype=np.float32)
    b2 = np.asarray(inputs["b2"], dtype=np.float32)
    es = np.asarray(inputs["edge_src"]).astype(np.int64)
    ed = np.asarray(inputs["edge_dst"]).astype(np.int64)

    npc = cfg.nodes_per_core
    nb = cfg.n_buckets
    T = tmpl.total_tiles

    table_bf16 = ego.astype(BF16)
    iota = np.broadcast_to(np.arange(P, dtype=np.float32), (P, P)).astype(BF16)
    iota = np.ascontiguousarray(iota)
    b1col = np.ascontiguousarray(b1[:, None])
    b2col = np.ascontiguousarray(b2[:, None])

    core_of_edge = ed // npc
    bucket_of_edge = es // cfg.bucket

    in_maps = []
    node_maps = []
    for c in range(cfg.n_cores):
        m = core_of_edge == c
        src_c = es[m]
        dstl_c = ed[m] - c * npc
        w_c = ew[m]
        bkt_c = bucket_of_edge[m]

        deg = np.bincount(dstl_c * nb + bkt_c, minlength=npc * nb).reshape(npc, nb)
        block_of = _greedy_pack(deg, cfg)
        if block_of is None:
            raise RuntimeError(f"packing failed for core {c} with caps {cfg.caps}")
        # slots within block: stable order by block
        order_nodes = np.argsort(block_of, kind="stable")
        slot_of = np.empty(npc, dtype=np.int64)
        blk_sorted = block_of[order_nodes]
        first_idx = np.searchsorted(blk_sorted, np.arange(cfg.n_blocks))
        slot_of[order_nodes] = np.arange(npc) - first_idx[blk_sorted]
        assert slot_of.max() < P

        node_map = np.full(cfg.n_blocks * P, -1, dtype=np.int64)
        node_map[block_of * P + slot_of] = np.arange(npc) + c * npc
        node_maps.append(node_map)

        # per-edge placement into the padded tile stream
        eblk = block_of[dstl_c]
        key = eblk * nb + bkt_c
        order_e = np.argsort(key, kind="stable")
        key_s = key[order_e]
        run_ids, run_firsts = np.unique(key_s, return_index=True)
        rank = np.arange(len(key_s))
        rank = rank - run_firsts[np.searchsorted(run_ids, key_s)]
        run_start_flat = np.full(cfg.n_blocks * nb, -1, dtype=np.int64)
        for (j, k), st in tmpl.run_start.items():
            run_start_flat[j * nb + k] = st
        counts = np.bincount(key_s, minlength=cfg.n_blocks * nb)
        caps_flat = np.array([cfg.cap(j, k) * P for j in range(cfg.n_blocks)
                              for k in range(nb)], dtype=np.int64)
        if (counts > caps_flat).any():
            bad = int(np.argmax(counts - caps_flat))
            raise RuntimeError(
                f"core {c}: run {bad // nb},{bad % nb} overflow "
                f"{counts[bad]} > {caps_flat[bad]}")
        pos = run_start_flat[key_s] * P + rank
        assert pos.max() < T * P

        idx_flat = np.zeros(T * P, dtype=np.int16)
        dst_flat = np.zeros(T * P, dtype=np.float32)
        w_flat = np.zeros(T * P, dtype=np.float32)
        idx_flat[pos] = (src_c[order_e] - bkt_c[order_e] * cfg.bucket).astype(np.int16)
        dst_flat[pos] = slot_of[dstl_c[order_e]].astype(np.float32)
        w_flat[pos] = w_c[order_e]

        idx_cols = idx_flat.reshape(-1, 16).T  # [16, T*8]
        idx_h = np.ascontiguousarray(np.tile(idx_cols, (8, 1)))  # [128, T*8]
        stream_h = np.ascontiguousarray(
            np.stack([dst_flat.reshape(T, P).T, w_flat.reshape(T, P).T],
                     axis=-1))  # [128, T, 2] f32
        if cfg.s2op:
            stream_h = stream_h.astype(BF16)

        egoT = np.zeros((P, cfg.n_blocks * P), dtype=np.float32)
        valid = node_map >= 0
        egoT[:, valid] = ego[node_map[valid]].T

        im = {
            "table": table_bf16,
            "idxs": idx_h,
            "stream": stream_h,
            "egoT": egoT,
            "w1": W1,   # [in=f, out] == lhsT [K=f, M=dout]
            "w2": W2,
            "iota": iota,
        }
        if cfg.with_bias:
            im["b1col"] = b1col
            im["b2col"] = b2col
        in_maps.append(im)
    return in_maps, node_maps


def assemble_output(results, node_maps, cfg: Cfg):
    y = np.zeros((cfg.n_nodes, D), dtype=np.float32)
    for c in range(cfg.n_cores):
        yT = results[c]["yT"]  # [128, n_blocks*P]
        nm = node_maps[c]
        valid = nm >= 0
        y[nm[valid]] = yT[:, valid].T
    return y


# ----------------------------------------------------------------------------
# device kernel
# ----------------------------------------------------------------------------
def _patch_sem_cleanup():
    """The walrus build in this container rejects the
    EVENT_SEMAPHORE_RANGE_CLEAR InstISA ("ISA wrong length") that
    TileContext emits on exit via Bass.clear_and_free_semaphores. The
    cleanup only matters for multi-iteration NEFFs (NRT re-initializes
    semaphores per execution), so skip the instruction emission and keep
    the allocator bookkeeping."""
    import concourse.bass as bass

    if getattr(bass.Bass, "_sem_cleanup_patched", False):
        return

    def patched(self, sems):
        if not sems:
            return
        sem_nums = [s.num if hasattr(s, "num") else s for s in sems]
        self._state.prepend_free_semaphores(sem_nums)
        for poison_set in self._tile_sem_poison_stack:
            poison_set.update(sem_nums)

    bass.Bass.clear_and_free_semaphores = patched
    bass.Bass._sem_cleanup_patched = True


_MANY_WAITS_OK = {
    "InstEventSemaphore",
}


def _split_excess_waits(nc, mybir, max_waits=1):
    """This container's walrus encodes at most `max_waits` sync-wait commands
    on TPB compute instructions ("Too many sync wait commands"). Hoist the
    excess onto EventSemaphore instructions inserted immediately before on
    the same engine — semantically identical (in-order sequencer stall)."""
    nid = 0
    for blk in nc.m.functions[0].blocks:
        il = blk.instructions
        i = 0
        while i < len(il):
            ins = il[i]
            si = ins.sync_info
            if (type(ins).__name__ not in _MANY_WAITS_OK and si is not None
                    and si.on_wait and len(si.on_wait) > max_waits):
                waits = list(si.on_wait)
                excess, keep = waits[:-max_waits], waits[-max_waits:]
                ins.sync_info = mybir.SyncInfo(
                    on_wait=keep, on_update=list(si.on_update or []))
                for w in excess:
                    es = mybir.InstEventSemaphore(
                        name=f"I-waitsplit-{nid}", engine=ins.engine,
                        ins=[], outs=[],
                        sync_info=mybir.SyncInfo(on_wait=[w], on_update=[]))
                    nid += 1
                    il.insert(i, es)
                    i += 1
            i += 1


def build_nc(cfg: Cfg, tmpl: Template):
    import concourse.bass as bass
    import concourse.mybir as mybir
    from concourse import library_config
    from concourse.tile import TileContext

    _patch_sem_cleanup()

    dt = mybir.dt
    T = tmpl.total_tiles
    NBLK = cfg.n_blocks
    sdt = dt.bfloat16 if cfg.s2op else dt.float32

    nc = bass.Bass(num_swdge_queues=4)
    table = nc.dram_tensor("table", [cfg.n_nodes, D], dt.bfloat16, kind="ExternalInput")
    idxs = nc.dram_tensor("idxs", [P, T * 8], dt.int16, kind="ExternalInput")
    stream = nc.dram_tensor("stream", [P, T, 2], sdt, kind="ExternalInput")
    egoT = nc.dram_tensor("egoT", [P, NBLK * P], dt.float32, kind="ExternalInput")
    wdt = dt.float32r if cfg.f32r else dt.float32
    w1 = nc.dram_tensor("w1", [D, D], wdt, kind="ExternalInput")
    w2 = nc.dram_tensor("w2", [D, D], wdt, kind="ExternalInput")
    iota = nc.dram_tensor("iota", [P, P], dt.bfloat16, kind="ExternalInput")
    if cfg.with_bias:
        b1col = nc.dram_tensor("b1col", [D, 1], dt.float32, kind="ExternalInput")
        b2col = nc.dram_tensor("b2col", [D, 1], dt.float32, kind="ExternalInput")
    yT = nc.dram_tensor("yT", [P, NBLK * P], dt.float32, kind="ExternalOutput")

    nc.gpsimd.load_library(library_config.mlp)

    with TileContext(nc) as tc:
        with (
            tc.tile_pool(name="const", bufs=1) as constp,
            tc.tile_pool(name="stage",
                         bufs=2 if cfg.s2w else (3 if cfg.deep_bufs else 2)) as stagep,
            tc.tile_pool(name="idxp", bufs=3 if cfg.deep_bufs else 2) as idxp,
            tc.tile_pool(name="streamp", bufs=3 if cfg.deep_bufs else 2) as streamp,
            tc.tile_pool(name="egop", bufs=3 if cfg.deep_bufs else 2) as egop,
            tc.tile_pool(name="sp", bufs=(2 if cfg.s2w else
                                          (16 if cfg.deep_bufs else 8))) as sp,
            tc.tile_pool(name="finp", bufs=(3 if cfg.quad_fin else
                                            (16 if cfg.deep_bufs else 12))) as finp,
            tc.tile_pool(name="outp", bufs=2) as outp,
            tc.tile_pool(name="accp", bufs=4, space="PSUM") as accp,
            tc.tile_pool(name="fpsum", bufs=2, space="PSUM") as fpsump,
        ):
            w1t = constp.tile([D, D], wdt)
            nc.sync.dma_start(out=w1t[:], in_=w1[:, :])
            w2t = constp.tile([D, D], wdt)
            nc.sync.dma_start(out=w2t[:], in_=w2[:, :])
            iotat = constp.tile([P, P], dt.bfloat16, name="iotat")
            nc.sync.dma_start(out=iotat[:], in_=iota[:, :])
            zerot = constp.tile([P, 4 * P], dt.bfloat16)
            nc.vector.memset(zerot[:], 0.0)
            if cfg.with_bias:
                b1t = constp.tile([D, 1], dt.float32)
                nc.sync.dma_start(out=b1t[:], in_=b1col[:, :])
                b2t = constp.tile([D, 1], dt.float32)
                nc.sync.dma_start(out=b2t[:], in_=b2col[:, :])

            w1mm, w2mm = w1t[:], w2t[:]

            gather_by_group = {}
            for (g, k, ts0, ntk) in tmpl.gathers:
                gather_by_group.setdefault(g, []).append((k, ts0, ntk))

            rounds = getattr(cfg, "rounds", 1)
            _gq = [0]  # global gather chunk counter for queue rotation
            # one register per distinct gather size (to_reg per call would
            # exhaust the Pool engine's register file)
            nidx_regs = {}
            for (_, _, _, ntk) in tmpl.gathers:
                n_idx = ntk * P
                if n_idx not in nidx_regs:
                    nidx_regs[n_idx] = nc.gpsimd.to_reg(n_idx)

            group_sched = [(r, g, bl) for r in range(rounds)
                           for g, bl in enumerate(tmpl.groups)]
            for _round, g, bl in group_sched:
                goff = tmpl.group_tile_off[g]
                gend = (tmpl.group_tile_off[g + 1]
                        if g + 1 < len(tmpl.groups) else T)
                ntg = gend - goff
                nblg = len(bl)

                idx_t = idxp.tile([P, ntg * 8], dt.int16, tag="idx")
                nc.sync.dma_start(out=idx_t[:], in_=idxs[:, goff * 8:gend * 8])
                str_t = streamp.tile([P, ntg, 2], sdt, tag="stream")
                nc.sync.dma_start(out=str_t[:], in_=stream[:, goff:gend, :])
                ego_t = egop.tile([P, nblg * P], dt.float32, tag="ego")
                nc.sync.dma_start(
                    out=ego_t[:], in_=egoT[:, bl[0] * P:(bl[-1] + 1) * P])

                stage_t = stagep.tile([P, ntg, D], dt.bfloat16, tag="stage")
                # Chunk each (group,bucket) gather and rotate the 4 SWDGE
                # queues (4 Q7 core pairs generate and drain descriptors
                # concurrently).
                glist = gather_by_group[g]
                # rotate issue order per group so desc-gen latency doesn't
                # always delay the same queue's drain
                glist = glist[g % len(glist):] + glist[:g % len(glist)]
                for (k, ts0, ntk) in glist:
                    blo = k * cfg.bucket
                    bhi = min((k + 1) * cfg.bucket, cfg.n_nodes)
                    done = 0
                    chunk = cfg.chunk_tiles if cfg.chunk_tiles > 0 else ntk
                    while done < ntk:
                        cn = min(chunk, ntk - done)
                        loc = ts0 - goff + done
                        n_idx = cn * P
                        if n_idx not in nidx_regs:
                            nidx_regs[n_idx] = nc.gpsimd.to_reg(n_idx)
                        if cfg.ablate_gather:
                            # same bytes via plain strided HWDGE copy:
                            # isolates SWDGE descriptor-gen cost
                            nc.sync.dma_start(
                                out=stage_t[:, loc:loc + cn, :],
                                in_=table[blo:blo + cn * P, :].rearrange(
                                    "(e t) d -> e t d", e=P))
                            done += cn
                            continue
                        nc.gpsimd.dma_gather(
                            stage_t[:, loc:loc + cn, :],
                            table[blo:bhi, :],
                            idx_t[:, loc * 8:(loc + cn) * 8],
                            n_idx,
                            nidx_regs[n_idx],
                            D,
                            single_packet=n_idx <= 1024,
                            queue_num=_gq[0] % 4,
                        )
                        _gq[0] += 1
                        done += cn

                # aggregation matmuls (stream order)
                n_quads = (nblg + 3) // 4
                quads = [accp.tile([P, 4, P], dt.float32, tag="acc",
                                   name=f"acc_g{g}_q{q}")
                         for q in range(n_quads)]
                # open each quad bank: one zeroing matmul covering all 4
                # block regions, so every later matmul is a plain accumulate
                # ordered after it (WAW on the full tile).
                for q in range(n_quads):
                    nc.tensor.matmul(
                        out=quads[q][:, :, :],
                        lhsT=iotat[:],
                        rhs=zerot[:],
                        start=True,
                        stop=False,
                        skip_group_check=True,
                    )

                def acc_ap(jj):
                    return quads[jj // 4][:, jj % 4, :]

                if cfg.s2w and not cfg.ablate_s:
                    # build S for the whole group in a few wide DVE op pairs
                    # (vs per-tile ops: kills per-tile PE sem waits and DVE
                    # seq overhead)
                    swide = sp.tile([P, ntg, P], dt.bfloat16, tag="Swide",
                                    name=f"swide_{_round}_{g}")
                    csz = (ntg + cfg.s_chunks - 1) // cfg.s_chunks
                    iota3 = iotat[:].rearrange("p (one d) -> p one d", one=1)
                    for h0 in range(0, ntg, csz):
                        h1 = min(h0 + csz, ntg)
                        nc.vector.tensor_tensor(
                            out=swide[:, h0:h1, :],
                            in0=iota3.to_broadcast([P, h1 - h0, P]),
                            in1=str_t[:, h0:h1, 0:1].to_broadcast(
                                [P, h1 - h0, P]),
                            op=mybir.AluOpType.is_equal)
                        nc.vector.tensor_tensor(
                            out=swide[:, h0:h1, :], in0=swide[:, h0:h1, :],
                            in1=str_t[:, h0:h1, 1:2].to_broadcast(
                                [P, h1 - h0, P]),
                            op=mybir.AluOpType.mult)

                for ti in range(goff, gend):
                    if cfg.ablate_mm:
                        break
                    j = tmpl.tile_block[ti]
                    jj = j - bl[0]
                    loc_t = ti - goff
                    if cfg.ablate_s:
                        S = iotat[:]
                    elif cfg.s2w:
                        S = swide[:, loc_t, :]
                    else:
                        S = sp.tile([P, P], dt.bfloat16, tag="S")
                        nc.vector.tensor_scalar(
                            out=S[:],
                            in0=iotat[:],
                            scalar1=str_t[:, loc_t, 0:1],
                            scalar2=str_t[:, loc_t, 1:2],
                            op0=mybir.AluOpType.is_equal,
                            op1=mybir.AluOpType.mult,
                        )
                    nc.tensor.matmul(
                        out=acc_ap(jj),
                        lhsT=stage_t[:, loc_t, :],
                        rhs=S[:],
                        start=False,
                        stop=tmpl.tile_last[ti],
                        skip_group_check=True,
                    )

                # finals
                out_t = outp.tile([P, nblg * P], dt.float32, tag="out")
                if cfg.quad_fin:
                    for q in range(n_quads):
                        qb = min(4, nblg - q * 4)
                        w = qb * P
                        a = quads[q].rearrange("p four d -> p (four d)")[:, :w]
                        ego_q = ego_t[:, q * 4 * P:q * 4 * P + w]
                        sumT = finp.tile([P, 4 * P], wdt, tag="sumT")
                        nc.vector.tensor_add(out=sumT[:, :w], in0=ego_q, in1=a)
                        biT = finp.tile([P, 4 * P], wdt, tag="biT")
                        nc.vector.tensor_tensor(
                            out=biT[:, :w], in0=ego_q, in1=a,
                            op=mybir.AluOpType.mult)
                        pp1 = fpsump.tile([P, 4 * P], dt.float32, tag="pp1")
                        nc.tensor.matmul(out=pp1[:, :w], lhsT=w1mm,
                                         rhs=sumT[:, :w],
                                         start=True, stop=True,
                                         skip_group_check=True)
                        pp2 = fpsump.tile([P, 4 * P], dt.float32, tag="pp2")
                        nc.tensor.matmul(out=pp2[:, :w], lhsT=w2mm,
                                         rhs=biT[:, :w],
                                         start=True, stop=True,
                                         skip_group_check=True)
                        l1 = finp.tile([P, 4 * P], dt.float32, tag="l1")
                        l2 = finp.tile([P, 4 * P], dt.float32, tag="l2")
                        if cfg.act_lrelu:
                            nc.scalar.activation(
                                out=l1[:, :w], in_=pp1[:, :w],
                                func=mybir.ActivationFunctionType.Lrelu,
                                bias=(b1t[:, 0:1] if cfg.with_bias else 0.0),
                                alpha=NEG_SLOPE)
                            nc.scalar.activation(
                                out=l2[:, :w], in_=pp2[:, :w],
                                func=mybir.ActivationFunctionType.Lrelu,
                                bias=(b2t[:, 0:1] if cfg.with_bias else 0.0),
                                alpha=NEG_SLOPE)
                        else:
                            for (pp, ll, bt) in ((pp1, l1, "b1"), (pp2, l2, "b2")):
                                if cfg.with_bias:
                                    z = finp.tile([P, 4 * P], dt.float32,
                                                  tag="z" + bt)
                                    nc.vector.tensor_scalar(
                                        out=z[:, :w], in0=pp[:, :w],
                                        scalar1=(b1t if bt == "b1" else b2t)[:, 0:1],
                                        scalar2=None,
                                        op0=mybir.AluOpType.add)
                                    src = z
                                else:
                                    src = pp
                                t_ = finp.tile([P, 4 * P], dt.float32,
                                               tag="t" + bt)
                                nc.vector.tensor_scalar(
                                    out=t_[:, :w], in0=src[:, :w],
                                    scalar1=NEG_SLOPE, scalar2=None,
                                    op0=mybir.AluOpType.mult)
                                nc.vector.tensor_tensor(
                                    out=ll[:, :w], in0=src[:, :w],
                                    in1=t_[:, :w], op=mybir.AluOpType.max)
                        nc.vector.tensor_add(
                            out=out_t[:, q * 4 * P:q * 4 * P + w],
                            in0=l1[:, :w], in1=l2[:, :w])
                else:
                    for jj, j in enumerate(bl):
                        a = acc_ap(jj)
                        ego_j = ego_t[:, jj * P:(jj + 1) * P]
                        sumT = finp.tile([P, P], dt.float32, tag="sumT")
                        nc.vector.tensor_add(out=sumT[:], in0=ego_j, in1=a)
                        biT = finp.tile([P, P], dt.float32, tag="biT")
                        nc.vector.tensor_tensor(
                            out=biT[:], in0=ego_j, in1=a,
                            op=mybir.AluOpType.mult)
                        pp1 = fpsump.tile([P, P], dt.float32, tag="pp1")
                        nc.tensor.matmul(out=pp1[:], lhsT=w1t[:], rhs=sumT[:],
                                         start=True, stop=True,
                                         skip_group_check=True)
                        pp2 = fpsump.tile([P, P], dt.float32, tag="pp2")
                        nc.tensor.matmul(out=pp2[:], lhsT=w2t[:], rhs=biT[:],
                                         start=True, stop=True,
                                         skip_group_check=True)
                        if cfg.with_bias:
                            z1 = finp.tile([P, P], dt.float32, tag="z1")
                            nc.vector.tensor_scalar(
                                out=z1[:], in0=pp1[:], scalar1=b1t[:, 0:1],
                                scalar2=None, op0=mybir.AluOpType.add)
                            z2 = finp.tile([P, P], dt.float32, tag="z2")
                            nc.vector.tensor_scalar(
                                out=z2[:], in0=pp2[:], scalar1=b2t[:, 0:1],
                                scalar2=None, op0=mybir.AluOpType.add)
                        else:
                            z1, z2 = pp1, pp2
                        t1 = finp.tile([P, P], dt.float32, tag="t1")
                        nc.vector.tensor_scalar(
                            out=t1[:], in0=z1[:], scalar1=NEG_SLOPE,
                            scalar2=None, op0=mybir.AluOpType.mult)
                        m1 = finp.tile([P, P], dt.float32, tag="m1")
                        nc.vector.tensor_tensor(
                            out=m1[:], in0=z1[:], in1=t1[:],
                            op=mybir.AluOpType.max)
                        t2 = finp.tile([P, P], dt.float32, tag="t2")
                        nc.vector.tensor_scalar(
                            out=t2[:], in0=z2[:], scalar1=NEG_SLOPE,
                            scalar2=None, op0=mybir.AluOpType.mult)
                        m2 = finp.tile([P, P], dt.float32, tag="m2")
                        nc.vector.tensor_tensor(
                            out=m2[:], in0=z2[:], in1=t2[:],
                            op=mybir.AluOpType.max)
                        nc.vector.tensor_add(
                            out=out_t[:, jj * P:(jj + 1) * P], in0=m1[:], in1=m2[:])

                nc.sync.dma_start(
                    out=yT[:, bl[0] * P:(bl[-1] + 1) * P], in_=out_t[:])

    return nc


def finalize_for_hw(nc):
    """Walrus-compat passes applied only on the compile path (CoreSim's
    race detector rejects the post-hoc EventSemaphore instructions)."""
    import concourse.mybir as mybir
    if getattr(nc, "_finalized_for_hw", False):
        return nc
    # raw-Bass doesn't run Bacc's codegen pass that fills in .instr bytes for
    # extended-ISA instructions (load_library pseudo); without it walrus
    # fails with "ISA wrong length".
    mybir.codegen_inst_isa_subclasses(nc)
    _split_excess_waits(nc, mybir)
    nc._finalized_for_hw = True
    return nc


# ----------------------------------------------------------------------------
# entry point
# ----------------------------------------------------------------------------
_CACHE = {}
LAST_EXEC_NS = None
TRACE = False

CAPS_LADDER = [(5, 4, 4, 4), (5, 5, 4, 4), (5, 5, 5, 4), (5, 5, 5, 5), (6, 5, 5, 5),
               (6, 6, 5, 5), (6, 6, 6, 6), (8, 8, 8, 8), (12, 12, 12, 12)]


def _get_compiled(cfg: Cfg):
    if cfg not in _CACHE:
        tmpl = build_template(cfg)
        nc = build_nc(cfg, tmpl)
        _CACHE[cfg] = (tmpl, nc)
    return _CACHE[cfg]


def kernel(**inputs) -> np.ndarray:
    global LAST_EXEC_NS
    with_bias = (np.any(np.asarray(inputs["b1"]) != 0)
                 or np.any(np.asarray(inputs["b2"]) != 0))
    chosen = None
    in_maps = node_maps = None
    for caps in CAPS_LADDER:
        trial = Cfg(caps=caps, with_bias=bool(with_bias))
        tmpl_try = build_template(trial)
        try:
            in_maps, node_maps = host_prep(inputs, trial, tmpl_try)
        except RuntimeError:
            continue
        chosen = trial
        break
    if chosen is None:
        raise RuntimeError("no feasible caps template")

    tmpl, nc = _get_compiled(chosen)
    finalize_for_hw(nc)

    from concourse.bass_utils import run_bass_kernel_spmd
    res = run_bass_kernel_spmd(
        nc, in_maps, core_ids=list(range(chosen.n_cores)), trace=TRACE)
    LAST_EXEC_NS = res.exec_time_ns
    return assemble_output(res.results, node_maps, chosen)
